# revision 11
# baseline (speedup 1.0000x reference)
"""Trainium2 Bass kernel for nn_CombinedLoss (3-branch local NCC loss).

Design: shard D=160 across 8 cores (20 interior slices each, 5-voxel halo,
host zero-padded to [30, 202, 170] per core). Per core, two 128-row H tiles.
Per branch (raw / Laplacian-edge / Sobel-magnitude): build fields, then for
each of the 5 NCC fields (A, B, A2, B2, AB) run the separable 9^3 box sum
as shift-add trees: D-axis tree (4 adds) -> W-axis tree (4 adds) -> H-axis
banded matmul on TensorE (3200 cols). Every tree level is split ~65/35
between the Vector (DVE) and GpSimd (Pool) engines along a seam-free axis;
squares / PSUM evacuation / reciprocal run on the Scalar (ACT) engine.
NCC pointwise math via fused scalar_tensor_tensor ops, reduced with
accum_out. Host combines the [128, 8] per-core partial sums.
"""
import numpy as np

N_CORES = 8
D, H, W = 160, 192, 160
DS = D // N_CORES          # 20
PAD = 5
DP = DS + 2 * PAD          # 30
HP = H + 2 * PAD           # 202
WP = W + 2 * PAD           # 170
INV_WS = float(np.float32(1.0 / 729.0))
EPS = 1e-5
NVOX = float(D * H * W)

# (h0, acc_lo, acc_hi, rlo, rhi) per H tile phase
H_TILES = [(0, 5, 101, 5, 127), (74, 27, 123, 1, 123)]

FR = 0.62                  # DVE share of each split elementwise op

_CACHE = {}


def _make_band(klo, khi):
    B = np.zeros((128, 128), np.float32)
    for r in range(128):
        for o in range(-4, 5):
            k = r + o
            if klo <= k < khi:
                B[k, r] = 1.0
    return B


def _build_program():
    import concourse.bass as bass
    import concourse.tile as tile
    from concourse import bacc, mybir

    f32 = mybir.dt.float32
    Alu = mybir.AluOpType
    Act = mybir.ActivationFunctionType
    nc = bacc.Bacc("TRN2", target_bir_lowering=False, debug=False,
                   num_devices=N_CORES)

    xt_d = nc.dram_tensor("xt", [DP, HP, WP], f32, kind="ExternalInput").ap()
    xp_d = nc.dram_tensor("xp", [DP, HP, WP], f32, kind="ExternalInput").ap()
    mk_d = nc.dram_tensor("mask", [128, 32], f32, kind="ExternalInput").ap()
    b0_d = nc.dram_tensor("band0", [128, 128], f32, kind="ExternalInput").ap()
    b1_d = nc.dram_tensor("band1", [128, 128], f32, kind="ExternalInput").ap()
    bl_d = nc.dram_tensor("band_lap", [128, 128], f32, kind="ExternalInput").ap()
    bs_d = nc.dram_tensor("band_121", [128, 128], f32, kind="ExternalInput").ap()
    bo_d = nc.dram_tensor("band_one", [128, 128], f32, kind="ExternalInput").ap()
    bd_d = nc.dram_tensor("band_drv", [128, 128], f32, kind="ExternalInput").ap()
    out_d = nc.dram_tensor("out", [128, 8], f32, kind="ExternalOutput").ap()

    with tile.TileContext(nc) as tc:
        with (
            tc.tile_pool(name="main", bufs=1) as pool,
            tc.tile_pool(name="psum", bufs=8, space="PSUM") as psum_pool,
        ):
            XT = pool.tile([128, DP * WP], f32, tag="XT")
            XP = pool.tile([128, DP * WP], f32, tag="XP")
            A = pool.tile([128, 29 * WP], f32, tag="A")
            B = pool.tile([128, 29 * WP], f32, tag="B")
            FSQ = pool.tile([128, 29 * WP], f32, tag="FSQ")
            T1 = pool.tile([128, 29 * WP], f32, tag="T1")
            T2 = pool.tile([128, 29 * WP], f32, tag="T2")
            EV = [pool.tile([128, 3200], f32, tag=f"EV{i}", name=f"EV{i}")
                  for i in range(4)]
            MK = pool.tile([128, 32], f32, tag="MK")
            BAND = [pool.tile([128, 128], f32, tag=f"BAND{i}", name=f"BAND{i}")
                    for i in range(6)]
            ACC = pool.tile([128, 8], f32, tag="ACC")

            def v3(t, d, w):
                return t[:].rearrange("p (d w) -> p d w", w=w)

            nc.sync.dma_start(MK[:], mk_d)
            for bt, bd in zip(BAND, [b0_d, b1_d, bl_d, bs_d, bo_d, bd_d]):
                nc.sync.dma_start(bt[:], bd)
            nc.vector.memset(A[:], 0.0)
            nc.vector.memset(B[:], 0.0)
            nc.vector.memset(FSQ[:], 0.0)
            nc.vector.memset(ACC[:], 0.0)

            XT3 = v3(XT, DP, WP)
            XP3 = v3(XP, DP, WP)
            A3 = v3(A, 29, WP)
            B3 = v3(B, 29, WP)
            FQ3 = v3(FSQ, 29, WP)
            T13 = v3(T1, 29, WP)
            T23 = v3(T2, 29, WP)
            # narrower views for the build helpers
            T1b = T1[:, 0:30 * 162].rearrange("p (d w) -> p d w", w=162)
            T2b = T2[:, 0:30 * 162].rearrange("p (d w) -> p d w", w=162)
            FTb = FSQ[:, 0:29 * 162].rearrange("p (d w) -> p d w", w=162)

            # ---- elementwise ops split across DVE / Pool -----------------
            # operand spec: (view, row_delta, col_delta) applied to the
            # common iteration box rows [d0:d1) x cols [w0:w1)
            def _sl(spec, r0, r1, c0, c1):
                v, dd, dw = spec
                return v[:, r0 + dd:r1 + dd, c0 + dw:c1 + dw]

            def _parts(d0, d1, w0, w1, axis):
                if axis == "w":
                    mid = w0 + max(1, int(round((w1 - w0) * FR)))
                    return ((nc.vector, d0, d1, w0, mid),
                            (nc.gpsimd, d0, d1, mid, w1))
                mid = d0 + max(1, int(round((d1 - d0) * FR)))
                return ((nc.vector, d0, mid, w0, w1),
                        (nc.gpsimd, mid, d1, w0, w1))

            def tt2(op, dst, a, b, d0, d1, w0, w1, axis):
                for eng, r0, r1, c0, c1 in _parts(d0, d1, w0, w1, axis):
                    getattr(eng, op)(_sl(dst, r0, r1, c0, c1),
                                     _sl(a, r0, r1, c0, c1),
                                     _sl(b, r0, r1, c0, c1))

            def stt2(dst, a, scal, b, op0, op1, d0, d1, w0, w1, axis):
                # scalar_tensor_tensor only exists on DVE (Pool ISA lacks
                # TensorScalarPtr) — run full-size on Vector
                nc.vector.scalar_tensor_tensor(
                    _sl(dst, d0, d1, w0, w1), _sl(a, d0, d1, w0, w1),
                    scal, _sl(b, d0, d1, w0, w1), op0, op1)

            def box(F3, band_t, ev_dst):
                D1, D2, DQ = (T13, 0, 0), (T23, 0, 0), (FQ3, 0, 0)
                F = (F3, 0, 0)
                # D-tree (full w; halo cols are zero in all sources)
                tt2("tensor_add", D1, F, (F3, 1, 0), 1, 28, 0, WP, "w")
                tt2("tensor_add", D2, D1, (T13, 2, 0), 1, 26, 0, WP, "w")
                tt2("tensor_add", D1, D2, (T23, 4, 0), 1, 22, 0, WP, "w")
                tt2("tensor_add", D2, D1, (F3, 8, 0), 1, 21, 0, WP, "w")
                # W-tree on s9 (=T2) rows 1..20
                tt2("tensor_add", D1, D2, (T23, 0, 1), 1, 21, 1, 168, "d")
                tt2("tensor_add", DQ, D1, (T13, 0, 2), 1, 21, 1, 166, "d")
                tt2("tensor_add", D1, DQ, (FQ3, 0, 4), 1, 21, 1, 162, "d")
                tt2("tensor_add", (FQ3, 0, -1), D1, (T23, 0, 8),
                    1, 21, 1, 161, "d")
                # H-axis banded matmul + ACT evacuation
                for c0 in range(1, 21, 3):
                    dc = min(3, 21 - c0)
                    ps = psum_pool.tile([128, 512], f32, tag="ps", name="ps")
                    nc.tensor.matmul(ps[:, 0:dc * 160], band_t[:],
                                     FQ3[:, c0:c0 + dc, 0:160],
                                     start=True, stop=True)
                    off = (c0 - 1) * 160
                    nc.scalar.copy(ev_dst[:, off:off + dc * 160],
                                   ps[:, 0:dc * 160])

            def mask_field(dst3):
                for dpad in list(range(1, 5)) + list(range(25, 29)):
                    nc.scalar.activation(
                        dst3[:, dpad:dpad + 1, 5:165],
                        dst3[:, dpad:dpad + 1, 5:165],
                        Act.Copy, scale=MK[:, dpad:dpad + 1])

            def build_E(X3, dst3):
                X = (X3, 0, 0)
                tt2("tensor_add", (T1b, 0, -5), (X3, 0, -1), (X3, 0, 1),
                    1, 29, 5, 165, "d")
                tt2("tensor_add", (T2b, 0, -5), (X3, -1, 0), (X3, 1, 0),
                    1, 29, 5, 165, "w")
                tt2("tensor_add", (FTb, 0, -5), (T1b, 0, -5), (T2b, 0, -5),
                    1, 29, 5, 165, "w")
                stt2((dst3, 0, 0), X, 4.0, (FTb, 0, -5),
                     Alu.mult, Alu.subtract, 1, 29, 5, 165, "w")
                for d0 in range(1, 29, 3):
                    dc = min(3, 29 - d0)
                    ps = psum_pool.tile([128, 512], f32, tag="ps", name="ps")
                    nc.tensor.matmul(ps[:, 0:dc * WP], BAND[2][:],
                                     X3[:, d0:d0 + dc, :], start=True, stop=True)
                    ps3 = ps[:, 0:dc * WP].rearrange("p (d w) -> p d w", w=WP)
                    nc.vector.tensor_add(dst3[:, d0:d0 + dc, 5:165],
                                         dst3[:, d0:d0 + dc, 5:165],
                                         ps3[:, :, 5:165])

            def g_mm_square(band_t, src3, dst3, first):
                # H-band matmul of src3 (field rows 1..28, cols = field w-5),
                # square PSUM, write/accumulate into dst3[:, 1:29, 5:165]
                for c0 in range(1, 29, 3):
                    cc = min(3, 29 - c0)
                    ps = psum_pool.tile([128, 512], f32, tag="ps", name="ps")
                    nc.tensor.matmul(ps[:, 0:cc * 160], band_t[:],
                                     src3[:, c0:c0 + cc, 0:160],
                                     start=True, stop=True)
                    ps3 = ps[:, 0:cc * 160].rearrange("p (d w) -> p d w", w=160)
                    if first:
                        nc.scalar.square(dst3[:, c0:c0 + cc, 5:165], ps3)
                    else:
                        nc.scalar.square(T2b[:, c0:c0 + cc, 0:160], ps3)
                        nc.gpsimd.tensor_add(dst3[:, c0:c0 + cc, 5:165],
                                             dst3[:, c0:c0 + cc, 5:165],
                                             T2b[:, c0:c0 + cc, 0:160])

            def build_S(X3, dst3):
                # gx = s121H(onesD(derivW))
                tt2("tensor_sub", (T1b, 0, -5), (X3, 0, 1), (X3, 0, -1),
                    0, 30, 5, 165, "d")
                tt2("tensor_add", (T2b, 0, -5), (T1b, -1, -5), (T1b, 1, -5),
                    1, 29, 5, 165, "w")
                tt2("tensor_add", (FTb, 0, -5), (T2b, 0, -5), (T1b, 0, -5),
                    1, 29, 5, 165, "w")
                g_mm_square(BAND[3], FTb, dst3, True)
                # gy = ones3H(s121W(derivD))
                tt2("tensor_sub", (T1b, 0, -4), (X3, 1, 0), (X3, -1, 0),
                    1, 29, 4, 166, "w")
                tt2("tensor_add", (T2b, 0, -5), (T1b, 0, -5), (T1b, 0, -3),
                    1, 29, 5, 165, "d")
                stt2((FTb, 0, -5), (T1b, 0, -4), 2.0, (T2b, 0, -5),
                     Alu.mult, Alu.add, 1, 29, 5, 165, "w")
                g_mm_square(BAND[4], FTb, dst3, False)
                # gz = derivH(s121D(onesW))
                tt2("tensor_add", (T1b, 0, -5), (X3, 0, -1), (X3, 0, 1),
                    0, 30, 5, 165, "d")
                tt2("tensor_add", (T2b, 0, -5), (T1b, 0, -5), (X3, 0, 0),
                    0, 30, 5, 165, "d")
                tt2("tensor_add", (FTb, 0, -5), (T2b, -1, -5), (T2b, 1, -5),
                    1, 29, 5, 165, "w")
                stt2((T1b, 0, -5), (T2b, 0, -5), 2.0, (FTb, 0, -5),
                     Alu.mult, Alu.add, 1, 29, 5, 165, "w")
                g_mm_square(BAND[5], T1b, dst3, False)
                nc.scalar.sqrt(dst3[:, 1:29, 5:165], dst3[:, 1:29, 5:165])

            stt_v = nc.vector.scalar_tensor_tensor
            stt_p = nc.gpsimd.scalar_tensor_tensor

            for ph, (h0, acc_lo, acc_hi, rlo, rhi) in enumerate(H_TILES):
                band = BAND[ph]
                nc.sync.dma_start(XT3, xt_d[:, h0:h0 + 128, :].transpose([1, 0, 2]))
                nc.sync.dma_start(XP3, xp_d[:, h0:h0 + 128, :].transpose([1, 0, 2]))

                for br in range(3):
                    if br == 0:
                        FA, FB = XT3, XP3
                    elif br == 1:
                        build_E(XT3, A3)
                        mask_field(A3)
                        build_E(XP3, B3)
                        mask_field(B3)
                        FA, FB = A3, B3
                    else:
                        build_S(XT3, A3)
                        mask_field(A3)
                        build_S(XP3, B3)
                        mask_field(B3)
                        FA, FB = A3, B3

                    box(FA, band, EV[0])                    # Is
                    box(FB, band, EV[1])                    # Js
                    nc.scalar.square(FQ3[:, 1:29, :], FA[:, 1:29, :])
                    box(FQ3, band, EV[2])                   # I2s
                    nc.scalar.square(FQ3[:, 1:29, :], FB[:, 1:29, :])
                    box(FQ3, band, EV[3])                   # J2s
                    tt2("tensor_mul", (FQ3, 0, 0), (FA, 0, 0), (FB, 0, 0),
                        1, 29, 0, WP, "w")
                    box(FQ3, band, T2)                      # IJs -> T2[0:3200]

                    # NCC pointwise on [128, 3200]
                    Is, Js, I2s, J2s = (e[:] for e in EV)
                    IJs = T2[:, 0:3200]
                    S1 = T1[:, 0:3200]
                    S2 = FSQ[:, 0:3200]
                    col = ACC[:, ph * 3 + br:ph * 3 + br + 1]
                    stt_v(S1, Is, INV_WS, Is, Alu.mult, Alu.mult)   # n2
                    nc.gpsimd.tensor_sub(I2s, I2s, S1)              # Ivar
                    stt_v(S2, Js, INV_WS, Js, Alu.mult, Alu.mult)   # n3
                    nc.gpsimd.tensor_sub(J2s, J2s, S2)              # Jvar
                    stt_v(S1, Is, INV_WS, Js, Alu.mult, Alu.mult)   # n1
                    nc.gpsimd.tensor_sub(IJs, IJs, S1)              # cross
                    stt_v(S1, I2s, EPS, J2s, Alu.add, Alu.mult)     # den
                    nc.vector.reciprocal_approx_fast(S2, S1)        # rden
                    nc.scalar.square(S1, IJs)                       # num
                    stt_v(Is, S1, 1.0, S2, Alu.mult, Alu.mult,
                          accum_out=col)

            nc.sync.dma_start(out_d, ACC[:])
    nc.compile()
    return nc


def _get_nc():
    if "nc" not in _CACHE:
        _CACHE["nc"] = _build_program()
    return _CACHE["nc"]


def _host_inputs(y_true, y_pred):
    xt = np.ascontiguousarray(np.asarray(y_true, np.float32).reshape(D, H, W))
    xp = np.ascontiguousarray(np.asarray(y_pred, np.float32).reshape(D, H, W))
    big_t = np.zeros((D + 2 * PAD, HP, WP), np.float32)
    big_p = np.zeros((D + 2 * PAD, HP, WP), np.float32)
    big_t[PAD:PAD + D, PAD:PAD + H, PAD:PAD + W] = xt
    big_p[PAD:PAD + D, PAD:PAD + H, PAD:PAD + W] = xp
    band0 = _make_band(5, 127)
    band1 = _make_band(1, 123)
    def bmat(taps):
        Bm = np.zeros((128, 128), np.float32)
        for o, t in taps:
            for r in range(128):
                if 0 <= r + o < 128:
                    Bm[r + o, r] += t
        return Bm
    band_lap = bmat([(-1, -1.0), (0, 2.0), (1, -1.0)])
    band_121 = bmat([(-1, 1.0), (0, 2.0), (1, 1.0)])
    band_one = bmat([(-1, 1.0), (0, 1.0), (1, 1.0)])
    band_drv = bmat([(-1, -1.0), (1, 1.0)])
    in_maps = []
    for c in range(N_CORES):
        d0 = c * DS
        mask = np.zeros((128, 32), np.float32)
        for j in range(DP):
            if 0 <= d0 - PAD + j < D:
                mask[:, j] = 1.0
        in_maps.append({
            "xt": np.ascontiguousarray(big_t[d0:d0 + DP]),
            "xp": np.ascontiguousarray(big_p[d0:d0 + DP]),
            "mask": mask,
            "band0": band0,
            "band1": band1,
            "band_lap": band_lap,
            "band_121": band_121,
            "band_one": band_one,
            "band_drv": band_drv,
        })
    return in_maps


def _combine(results):
    total = np.zeros(3, np.float64)
    for res in results:
        cols = np.asarray(res["out"], np.float64)
        for ph, (_, lo, hi, _, _) in enumerate(H_TILES):
            for br in range(3):
                total[br] += cols[lo:hi, ph * 3 + br].sum()
    losses = -total / NVOX
    return np.float32(0.8 * losses[0] + 0.1 * losses[1] + 0.1 * losses[2])


def kernel(y_true, y_pred):
    from concourse.bass_utils import run_bass_kernel_spmd
    nc = _get_nc()
    in_maps = _host_inputs(y_true, y_pred)
    res = run_bass_kernel_spmd(nc, in_maps, core_ids=list(range(N_CORES)))
    return _combine(res.results)


if __name__ == "__main__":
    g = np.load("/root/problem/golden.npz")
    got = float(kernel(g["y_true"], g["y_pred"]))
    exp = float(g["expected"])
    print(f"expected {exp:.9f} got {got:.9f} rel {abs(got-exp)/abs(exp):.3e}")


# revision 12
# speedup vs baseline: 1.9288x; 1.9288x over previous
"""Trainium2 Bass kernel for nn_CombinedLoss (3-branch local NCC loss).

Design: shard D=160 across 8 cores (20 interior slices each, 5-voxel halo,
host zero-padded fp16 [30, 202, 170] per core). Per core, two 128-row H
tiles. Per branch (raw / Laplacian-edge / Sobel-magnitude): build fields,
then for each of the 5 NCC fields (A, B, A2, B2, AB) run the separable 9^3
box sum as fp16 shift-add trees on DVE (flat contiguous ops, 2x mode):
D-axis tree (4 adds) -> W-axis tree (4 adds) -> H-axis banded matmul on
TensorE (fp16 moving, 3200 cols) -> fp32 PSUM evacuated by the ACT engine.
NCC pointwise math in fp32 via fused scalar_tensor_tensor ops (subs on the
Pool engine), reduced with accum_out. Host combines [128, 8] partials.
"""
import numpy as np

N_CORES = 8
D, H, W = 160, 192, 160
DS = D // N_CORES          # 20
PAD = 5
DP = DS + 2 * PAD          # 30
HP = H + 2 * PAD           # 202
WP = W + 2 * PAD           # 170
INV_WS = float(np.float32(1.0 / 729.0))
EPS = 1e-5
NVOX = float(D * H * W)

H_TILES = [(0, 5, 101, 5, 127), (74, 27, 123, 1, 123)]

_CACHE = {}


def _make_band(klo, khi):
    B = np.zeros((128, 128), np.float32)
    for r in range(128):
        for o in range(-4, 5):
            k = r + o
            if klo <= k < khi:
                B[k, r] = 1.0
    return B


def _build_program():
    import concourse.bass as bass
    import concourse.tile as tile
    from concourse import bacc, mybir

    f32 = mybir.dt.float32
    f16 = mybir.dt.float16
    Alu = mybir.AluOpType
    Act = mybir.ActivationFunctionType
    nc = bacc.Bacc("TRN2", target_bir_lowering=False, debug=False,
                   num_devices=N_CORES)

    xt_d = nc.dram_tensor("xt", [DP, HP, WP], f16, kind="ExternalInput").ap()
    xp_d = nc.dram_tensor("xp", [DP, HP, WP], f16, kind="ExternalInput").ap()
    mk_d = nc.dram_tensor("mask", [128, 32], f32, kind="ExternalInput").ap()
    b0_d = nc.dram_tensor("band0", [128, 128], f16, kind="ExternalInput").ap()
    b1_d = nc.dram_tensor("band1", [128, 128], f16, kind="ExternalInput").ap()
    bl_d = nc.dram_tensor("band_lap", [128, 128], f16, kind="ExternalInput").ap()
    bs_d = nc.dram_tensor("band_121", [128, 128], f16, kind="ExternalInput").ap()
    bo_d = nc.dram_tensor("band_one", [128, 128], f16, kind="ExternalInput").ap()
    bd_d = nc.dram_tensor("band_drv", [128, 128], f16, kind="ExternalInput").ap()
    out_d = nc.dram_tensor("out", [128, 8], f32, kind="ExternalOutput").ap()

    with tile.TileContext(nc) as tc:
        with (
            tc.tile_pool(name="main", bufs=1) as pool,
            tc.tile_pool(name="psum", bufs=8, space="PSUM") as psum_pool,
        ):
            XT = pool.tile([128, DP * WP], f16, tag="XT")
            XP = pool.tile([128, DP * WP], f16, tag="XP")
            A = pool.tile([128, 29 * WP], f16, tag="A")
            B = pool.tile([128, 29 * WP], f16, tag="B")
            FSQ = pool.tile([128, 29 * WP], f16, tag="FSQ")
            T1 = pool.tile([128, 30 * WP], f16, tag="T1")
            T2 = pool.tile([128, 30 * WP], f16, tag="T2")
            EV = [pool.tile([128, 3200], f32, tag=f"EV{i}", name=f"EV{i}")
                  for i in range(5)]
            PW1 = pool.tile([128, 3200], f32, tag="PW1")
            PW2 = pool.tile([128, 3200], f32, tag="PW2")
            MK = pool.tile([128, 32], f32, tag="MK")
            BAND = [pool.tile([128, 128], f16, tag=f"BAND{i}", name=f"BAND{i}")
                    for i in range(6)]
            ACC = pool.tile([128, 8], f32, tag="ACC")

            def v3(t, d, w=WP):
                return t[:].rearrange("p (d w) -> p d w", w=w)

            nc.sync.dma_start(MK[:], mk_d)
            for bt, bd in zip(BAND, [b0_d, b1_d, bl_d, bs_d, bo_d, bd_d]):
                nc.sync.dma_start(bt[:], bd)
            nc.vector.memset(A[:], 0.0)
            nc.vector.memset(B[:], 0.0)
            nc.vector.memset(FSQ[:], 0.0)
            nc.vector.memset(T1[:], 0.0)
            nc.vector.memset(T2[:], 0.0)
            nc.vector.memset(ACC[:], 0.0)

            XT3, XP3 = v3(XT, DP), v3(XP, DP)
            A3, B3, FQ3 = v3(A, 29), v3(B, 29), v3(FSQ, 29)
            T13, T23 = v3(T1, 30), v3(T2, 30)

            VA = nc.vector.tensor_add
            VS = nc.vector.tensor_sub
            VM = nc.vector.tensor_mul
            stt = nc.vector.scalar_tensor_tensor

            def box(Ft, band_t, ev_dst):
                """Ft: fp16 field tile (rows d at flat d*170, needs rows
                1..28 valid, halo cols zero). 9^3 box -> ev_dst fp32."""
                Ff = Ft[:]
                T1f, T2f, FQf = T1[:], T2[:], FSQ[:]
                VA(T1f[:, 170:4760], Ff[:, 170:4760], Ff[:, 340:4930])  # t1
                VA(T2f[:, 170:4420], T1f[:, 170:4420], T1f[:, 510:4760])
                VA(T1f[:, 170:3740], T2f[:, 170:3740], T2f[:, 850:4420])
                VA(T2f[:, 170:3570], T1f[:, 170:3570], Ff[:, 1530:4930])  # s9
                VA(T1f[:, 170:3570], T2f[:, 170:3570], T2f[:, 171:3571])  # u1
                VA(FQf[:, 170:3570], T1f[:, 170:3570], T1f[:, 172:3572])  # u2
                VA(T1f[:, 170:3570], FQf[:, 170:3570], FQf[:, 174:3574])  # u4
                VA(FQf[:, 170:3570], T1f[:, 171:3571], T2f[:, 179:3579])  # v9
                for c0 in range(1, 21, 3):
                    dc = min(3, 21 - c0)
                    ps = psum_pool.tile([128, 512], f32, tag="ps", name="ps")
                    nc.tensor.matmul(ps[:, 0:dc * 160], band_t[:],
                                     FQ3[:, c0:c0 + dc, 0:160],
                                     start=True, stop=True)
                    off = (c0 - 1) * 160
                    nc.scalar.copy(ev_dst[:, off:off + dc * 160],
                                   ps[:, 0:dc * 160])

            def mask_field(dst3):
                for dpad in list(range(1, 5)) + list(range(25, 29)):
                    nc.scalar.activation(
                        dst3[:, dpad:dpad + 1, 5:165],
                        dst3[:, dpad:dpad + 1, 5:165],
                        Act.Copy, scale=MK[:, dpad:dpad + 1])

            def build_E(Xt, X3, dst3):
                Xf, T1f, T2f, FQf = Xt[:], T1[:], T2[:], FSQ[:]
                # w-pair, d-pair, sum (flat full-width; edge cols poison)
                VA(T1f[:, 170:4930], Xf[:, 169:4929], Xf[:, 171:4931])
                VA(T2f[:, 170:4930], Xf[:, 0:4760], Xf[:, 340:5100])
                VA(FQf[:, 170:4930], T1f[:, 170:4930], T2f[:, 170:4930])
                stt(dst3[:, 1:29, 5:165], X3[:, 1:29, 5:165], 4.0,
                    FQ3[:, 1:29, 5:165], Alu.mult, Alu.subtract)
                # H part: band_lap matmul, ACT-copy psum to T1 (fp16), 1 add
                for d0 in range(1, 29, 3):
                    dc = min(3, 29 - d0)
                    ps = psum_pool.tile([128, 512], f32, tag="ps", name="ps")
                    nc.tensor.matmul(ps[:, 0:dc * WP], BAND[2][:],
                                     X3[:, d0:d0 + dc, :], start=True, stop=True)
                    nc.scalar.copy(T1f[:, d0 * WP:(d0 + dc) * WP],
                                   ps[:, 0:dc * WP])
                VA(dst3[:, 1:29, 5:165], dst3[:, 1:29, 5:165],
                   T13[:, 1:29, 5:165])

            def g_mm_square(band_t, src3, dst3, first):
                # H-band matmul of src3 (rows 1..28, field cols 5..165),
                # square PSUM -> dst3 (first) or T2 then one add
                for c0 in range(1, 29, 3):
                    cc = min(3, 29 - c0)
                    ps = psum_pool.tile([128, 512], f32, tag="ps", name="ps")
                    nc.tensor.matmul(ps[:, 0:cc * 160], band_t[:],
                                     src3[:, c0:c0 + cc, 5:165],
                                     start=True, stop=True)
                    ps3 = ps[:, 0:cc * 160].rearrange("p (d w) -> p d w", w=160)
                    if first:
                        nc.scalar.square(dst3[:, c0:c0 + cc, 5:165], ps3)
                    else:
                        nc.scalar.square(T23[:, c0:c0 + cc, 5:165], ps3)
                if not first:
                    VA(dst3[:, 1:29, 5:165], dst3[:, 1:29, 5:165],
                       T23[:, 1:29, 5:165])

            def build_S(Xt, X3, dst3):
                Xf, T1f, T2f, FQf = Xt[:], T1[:], T2[:], FSQ[:]
                # gx = s121H(onesD(derivW))
                VS(T1f[:, 1:5099], Xf[:, 2:5100], Xf[:, 0:5098])
                VA(T2f[:, 170:4930], T1f[:, 0:4760], T1f[:, 340:5100])
                VA(FQf[:, 170:4930], T2f[:, 170:4930], T1f[:, 170:4930])
                g_mm_square(BAND[3], FQ3, dst3, True)
                # gy = ones3H(s121W(derivD))
                VS(T1f[:, 170:4930], Xf[:, 340:5100], Xf[:, 0:4760])
                VA(T2f[:, 171:4929], T1f[:, 170:4928], T1f[:, 172:4930])
                stt(FQf[:, 170:4930], T1f[:, 170:4930], 2.0,
                    T2f[:, 170:4930], Alu.mult, Alu.add)
                g_mm_square(BAND[4], FQ3, dst3, False)
                # gz = derivH(s121D(onesW))
                VA(T1f[:, 1:5099], Xf[:, 0:5098], Xf[:, 2:5100])
                VA(T2f[:, 1:5099], T1f[:, 1:5099], Xf[:, 1:5099])
                VA(FQf[:, 170:4930], T2f[:, 0:4760], T2f[:, 340:5100])
                stt(T1f[:, 170:4930], T2f[:, 170:4930], 2.0,
                    FQf[:, 170:4930], Alu.mult, Alu.add)
                g_mm_square(BAND[5], T13, dst3, False)
                nc.scalar.sqrt(dst3[:, 1:29, 5:165], dst3[:, 1:29, 5:165])

            for ph, (h0, acc_lo, acc_hi, rlo, rhi) in enumerate(H_TILES):
                band = BAND[ph]
                nc.sync.dma_start(XT3, xt_d[:, h0:h0 + 128, :].transpose([1, 0, 2]))
                nc.sync.dma_start(XP3, xp_d[:, h0:h0 + 128, :].transpose([1, 0, 2]))

                for br in range(3):
                    if br == 0:
                        FAt, FA3, FBt, FB3 = XT, XT3, XP, XP3
                    elif br == 1:
                        build_E(XT, XT3, A3)
                        mask_field(A3)
                        build_E(XP, XP3, B3)
                        mask_field(B3)
                        FAt, FA3, FBt, FB3 = A, A3, B, B3
                    else:
                        build_S(XT, XT3, A3)
                        mask_field(A3)
                        build_S(XP, XP3, B3)
                        mask_field(B3)
                        FAt, FA3, FBt, FB3 = A, A3, B, B3

                    box(FAt, band, EV[0][:])                # Is
                    box(FBt, band, EV[1][:])                # Js
                    nc.scalar.square(FQ3[:, 1:29, :], FA3[:, 1:29, :])
                    box(FSQ, band, EV[2][:])                # I2s
                    nc.scalar.square(FQ3[:, 1:29, :], FB3[:, 1:29, :])
                    box(FSQ, band, EV[3][:])                # J2s
                    VM(FQ3[:, 1:29, :], FA3[:, 1:29, :], FB3[:, 1:29, :])
                    box(FSQ, band, EV[4][:])                # IJs

                    # NCC pointwise on [128, 3200] fp32
                    Is, Js, I2s, J2s, IJs = (e[:] for e in EV)
                    S1, S2 = PW1[:], PW2[:]
                    col = ACC[:, ph * 3 + br:ph * 3 + br + 1]
                    stt(S1, Is, INV_WS, Is, Alu.mult, Alu.mult)     # n2
                    nc.gpsimd.tensor_sub(I2s, I2s, S1)              # Ivar
                    stt(S2, Js, INV_WS, Js, Alu.mult, Alu.mult)     # n3
                    nc.gpsimd.tensor_sub(J2s, J2s, S2)              # Jvar
                    stt(S1, Is, INV_WS, Js, Alu.mult, Alu.mult)     # n1
                    nc.gpsimd.tensor_sub(IJs, IJs, S1)              # cross
                    stt(S1, I2s, EPS, J2s, Alu.add, Alu.mult)       # den
                    nc.vector.reciprocal_approx_fast(S2, S1)        # rden
                    nc.scalar.square(S1, IJs)                       # num
                    stt(Is, S1, 1.0, S2, Alu.mult, Alu.mult,
                        accum_out=col)

            nc.sync.dma_start(out_d, ACC[:])
    nc.compile()
    return nc


def _get_nc():
    if "nc" not in _CACHE:
        _CACHE["nc"] = _build_program()
    return _CACHE["nc"]


def _host_inputs(y_true, y_pred):
    xt = np.ascontiguousarray(np.asarray(y_true, np.float32).reshape(D, H, W))
    xp = np.ascontiguousarray(np.asarray(y_pred, np.float32).reshape(D, H, W))
    big_t = np.zeros((D + 2 * PAD, HP, WP), np.float16)
    big_p = np.zeros((D + 2 * PAD, HP, WP), np.float16)
    big_t[PAD:PAD + D, PAD:PAD + H, PAD:PAD + W] = xt.astype(np.float16)
    big_p[PAD:PAD + D, PAD:PAD + H, PAD:PAD + W] = xp.astype(np.float16)
    band0 = _make_band(5, 127).astype(np.float16)
    band1 = _make_band(1, 123).astype(np.float16)
    def bmat(taps):
        Bm = np.zeros((128, 128), np.float32)
        for o, t in taps:
            for r in range(128):
                if 0 <= r + o < 128:
                    Bm[r + o, r] += t
        return Bm.astype(np.float16)
    band_lap = bmat([(-1, -1.0), (0, 2.0), (1, -1.0)])
    band_121 = bmat([(-1, 1.0), (0, 2.0), (1, 1.0)])
    band_one = bmat([(-1, 1.0), (0, 1.0), (1, 1.0)])
    band_drv = bmat([(-1, -1.0), (1, 1.0)])
    in_maps = []
    for c in range(N_CORES):
        d0 = c * DS
        mask = np.zeros((128, 32), np.float32)
        for j in range(DP):
            if 0 <= d0 - PAD + j < D:
                mask[:, j] = 1.0
        in_maps.append({
            "xt": np.ascontiguousarray(big_t[d0:d0 + DP]),
            "xp": np.ascontiguousarray(big_p[d0:d0 + DP]),
            "mask": mask,
            "band0": band0,
            "band1": band1,
            "band_lap": band_lap,
            "band_121": band_121,
            "band_one": band_one,
            "band_drv": band_drv,
        })
    return in_maps


def _combine(results):
    total = np.zeros(3, np.float64)
    for res in results:
        cols = np.asarray(res["out"], np.float64)
        for ph, (_, lo, hi, _, _) in enumerate(H_TILES):
            for br in range(3):
                total[br] += cols[lo:hi, ph * 3 + br].sum()
    losses = -total / NVOX
    return np.float32(0.8 * losses[0] + 0.1 * losses[1] + 0.1 * losses[2])


def kernel(y_true, y_pred):
    from concourse.bass_utils import run_bass_kernel_spmd
    nc = _get_nc()
    in_maps = _host_inputs(y_true, y_pred)
    res = run_bass_kernel_spmd(nc, in_maps, core_ids=list(range(N_CORES)))
    return _combine(res.results)


if __name__ == "__main__":
    g = np.load("/root/problem/golden.npz")
    got = float(kernel(g["y_true"], g["y_pred"]))
    exp = float(g["expected"])
    print(f"expected {exp:.9f} got {got:.9f} rel {abs(got-exp)/abs(exp):.3e}")


# revision 13
# speedup vs baseline: 2.0919x; 1.0846x over previous
"""Trainium2 Bass kernel for nn_CombinedLoss (3-branch local NCC loss).

Design: shard D=160 across 8 cores (20 interior slices each, 5-voxel halo,
host zero-padded fp16 [30, 202, 170] per core). Per core, two 128-row H
tiles. Per branch (raw / Laplacian-edge / Sobel-magnitude): build fields,
then for each of the 5 NCC fields (A, B, A2, B2, AB) run the separable 9^3
box sum as fp16 shift-add trees on DVE (flat contiguous ops, 2x mode):
D-axis tree (4 adds) -> W-axis tree (4 adds) -> H-axis banded matmul on
TensorE (fp16 moving, 3200 cols) -> fp32 PSUM evacuated by the ACT engine.
NCC pointwise math in fp32 via fused scalar_tensor_tensor ops (subs on the
Pool engine), reduced with accum_out. Host combines [128, 8] partials.
"""
import numpy as np

N_CORES = 8
D, H, W = 160, 192, 160
DS = D // N_CORES          # 20
PAD = 5
DP = DS + 2 * PAD          # 30
HP = H + 2 * PAD           # 202
WP = W + 2 * PAD           # 170
INV_WS = float(np.float32(1.0 / 729.0))
EPS = 1e-5
NVOX = float(D * H * W)

H_TILES = [(0, 5, 101, 5, 127), (74, 27, 123, 1, 123)]

_CACHE = {}


def _make_band(klo, khi):
    B = np.zeros((128, 128), np.float32)
    for r in range(128):
        for o in range(-4, 5):
            k = r + o
            if klo <= k < khi:
                B[k, r] = 1.0
    return B


def _build_program():
    import concourse.bass as bass
    import concourse.tile as tile
    from concourse import bacc, mybir

    f32 = mybir.dt.float32
    f16 = mybir.dt.float16
    Alu = mybir.AluOpType
    Act = mybir.ActivationFunctionType
    nc = bacc.Bacc("TRN2", target_bir_lowering=False, debug=False,
                   num_devices=N_CORES)

    xt_d = nc.dram_tensor("xt", [DP, HP, WP], f16, kind="ExternalInput").ap()
    xp_d = nc.dram_tensor("xp", [DP, HP, WP], f16, kind="ExternalInput").ap()
    mk_d = nc.dram_tensor("mask", [128, 32], f32, kind="ExternalInput").ap()
    b0_d = nc.dram_tensor("band0", [128, 128], f16, kind="ExternalInput").ap()
    b1_d = nc.dram_tensor("band1", [128, 128], f16, kind="ExternalInput").ap()
    bl_d = nc.dram_tensor("band_lap", [128, 128], f16, kind="ExternalInput").ap()
    bs_d = nc.dram_tensor("band_121", [128, 128], f16, kind="ExternalInput").ap()
    bo_d = nc.dram_tensor("band_one", [128, 128], f16, kind="ExternalInput").ap()
    bd_d = nc.dram_tensor("band_drv", [128, 128], f16, kind="ExternalInput").ap()
    out_d = nc.dram_tensor("out", [128, 8], f32, kind="ExternalOutput").ap()

    with tile.TileContext(nc) as tc:
        with (
            tc.tile_pool(name="main", bufs=1) as pool,
            tc.tile_pool(name="psum", bufs=8, space="PSUM") as psum_pool,
        ):
            XT = pool.tile([128, DP * WP], f16, tag="XT")
            XP = pool.tile([128, DP * WP], f16, tag="XP")
            A = pool.tile([128, 29 * WP], f16, tag="A")
            B = pool.tile([128, 29 * WP], f16, tag="B")
            FSQ = pool.tile([128, 29 * WP], f16, tag="FSQ")
            T1 = pool.tile([128, 30 * WP], f16, tag="T1")
            T2 = pool.tile([128, 30 * WP], f16, tag="T2")
            EV = [pool.tile([128, 3200], f16, tag=f"EV{i}", name=f"EV{i}")
                  for i in range(5)]
            PW1 = pool.tile([128, 3200], f32, tag="PW1")
            PW2 = pool.tile([128, 3200], f32, tag="PW2")
            PWH = pool.tile([128, 3200], f16, tag="PWH")
            MK = pool.tile([128, 32], f32, tag="MK")
            BAND = [pool.tile([128, 128], f16, tag=f"BAND{i}", name=f"BAND{i}")
                    for i in range(6)]
            ACC = pool.tile([128, 8], f32, tag="ACC")

            def v3(t, d, w=WP):
                return t[:].rearrange("p (d w) -> p d w", w=w)

            nc.sync.dma_start(MK[:], mk_d)
            for bt, bd in zip(BAND, [b0_d, b1_d, bl_d, bs_d, bo_d, bd_d]):
                nc.sync.dma_start(bt[:], bd)
            nc.vector.memset(A[:], 0.0)
            nc.vector.memset(B[:], 0.0)
            nc.vector.memset(FSQ[:], 0.0)
            nc.vector.memset(T1[:], 0.0)
            nc.vector.memset(T2[:], 0.0)
            nc.vector.memset(ACC[:], 0.0)

            XT3, XP3 = v3(XT, DP), v3(XP, DP)
            A3, B3, FQ3 = v3(A, 29), v3(B, 29), v3(FSQ, 29)
            T13, T23 = v3(T1, 30), v3(T2, 30)

            VA = nc.vector.tensor_add
            VS = nc.vector.tensor_sub
            VM = nc.vector.tensor_mul
            stt = nc.vector.scalar_tensor_tensor

            def box(Ft, band_t, ev_dst):
                """Ft: fp16 field tile (rows d at flat d*170, needs rows
                1..28 valid, halo cols zero). 9^3 box -> ev_dst fp32."""
                Ff = Ft[:]
                T1f, T2f, FQf = T1[:], T2[:], FSQ[:]
                VA(T1f[:, 170:4760], Ff[:, 170:4760], Ff[:, 340:4930])  # t1
                VA(T2f[:, 170:4420], T1f[:, 170:4420], T1f[:, 510:4760])
                VA(T1f[:, 170:3740], T2f[:, 170:3740], T2f[:, 850:4420])
                VA(T2f[:, 170:3570], T1f[:, 170:3570], Ff[:, 1530:4930])  # s9
                VA(T1f[:, 170:3570], T2f[:, 170:3570], T2f[:, 171:3571])  # u1
                VA(FQf[:, 170:3570], T1f[:, 170:3570], T1f[:, 172:3572])  # u2
                VA(T1f[:, 170:3570], FQf[:, 170:3570], FQf[:, 174:3574])  # u4
                VA(FQf[:, 170:3570], T1f[:, 171:3571], T2f[:, 179:3579])  # v9
                for c0 in range(1, 21, 3):
                    dc = min(3, 21 - c0)
                    ps = psum_pool.tile([128, 512], f32, tag="ps", name="ps")
                    nc.tensor.matmul(ps[:, 0:dc * 160], band_t[:],
                                     FQ3[:, c0:c0 + dc, 0:160],
                                     start=True, stop=True)
                    off = (c0 - 1) * 160
                    nc.scalar.copy(ev_dst[:, off:off + dc * 160],
                                   ps[:, 0:dc * 160])

            def mask_field(dst3):
                for dpad in list(range(1, 5)) + list(range(25, 29)):
                    nc.scalar.activation(
                        dst3[:, dpad:dpad + 1, 5:165],
                        dst3[:, dpad:dpad + 1, 5:165],
                        Act.Copy, scale=MK[:, dpad:dpad + 1])

            def zap_halo(dst3):
                nc.vector.memset(dst3[:, 1:29, 0:5], 0.0)
                nc.vector.memset(dst3[:, 1:29, 165:170], 0.0)

            def build_E(Xt, X3, dstt, dst3):
                Xf, T1f, T2f, FQf, Df = Xt[:], T1[:], T2[:], FSQ[:], dstt[:]
                # w-pair, d-pair, sum (flat full-width; edge cols poison)
                VA(T1f[:, 170:4930], Xf[:, 169:4929], Xf[:, 171:4931])
                VA(T2f[:, 170:4930], Xf[:, 0:4760], Xf[:, 340:5100])
                VA(FQf[:, 170:4930], T1f[:, 170:4930], T2f[:, 170:4930])
                stt(Df[:, 170:4930], Xf[:, 170:4930], 4.0,
                    FQf[:, 170:4930], Alu.mult, Alu.subtract)
                # H part: band_lap matmul, ACT-copy psum to T1 (fp16), 1 add
                for d0 in range(1, 29, 3):
                    dc = min(3, 29 - d0)
                    ps = psum_pool.tile([128, 512], f32, tag="ps", name="ps")
                    nc.tensor.matmul(ps[:, 0:dc * WP], BAND[2][:],
                                     X3[:, d0:d0 + dc, :], start=True, stop=True)
                    nc.scalar.copy(T1f[:, d0 * WP:(d0 + dc) * WP],
                                   ps[:, 0:dc * WP])
                VA(Df[:, 170:4930], Df[:, 170:4930], T1f[:, 170:4930])

            def g_mm_square(band_t, src3, dstt, first):
                # H-band matmul of src3 (rows 1..28, full width), square
                # PSUM -> dst flat (first) or T2 flat then one flat add
                Df, T2f = dstt[:], T2[:]
                for c0 in range(1, 29, 3):
                    cc = min(3, 29 - c0)
                    ps = psum_pool.tile([128, 512], f32, tag="ps", name="ps")
                    nc.tensor.matmul(ps[:, 0:cc * WP], band_t[:],
                                     src3[:, c0:c0 + cc, :],
                                     start=True, stop=True)
                    if first:
                        nc.scalar.square(Df[:, c0 * WP:(c0 + cc) * WP],
                                         ps[:, 0:cc * WP])
                    else:
                        nc.scalar.square(T2f[:, c0 * WP:(c0 + cc) * WP],
                                         ps[:, 0:cc * WP])
                if not first:
                    VA(Df[:, 170:4930], Df[:, 170:4930], T2f[:, 170:4930])

            def build_S(Xt, X3, dstt, dst3):
                Xf, T1f, T2f, FQf, Df = Xt[:], T1[:], T2[:], FSQ[:], dstt[:]
                # gx = s121H(onesD(derivW))
                VS(T1f[:, 1:5099], Xf[:, 2:5100], Xf[:, 0:5098])
                VA(T2f[:, 170:4930], T1f[:, 0:4760], T1f[:, 340:5100])
                VA(FQf[:, 170:4930], T2f[:, 170:4930], T1f[:, 170:4930])
                g_mm_square(BAND[3], FQ3, dstt, True)
                # gy = ones3H(s121W(derivD))
                VS(T1f[:, 170:4930], Xf[:, 340:5100], Xf[:, 0:4760])
                VA(T2f[:, 171:4929], T1f[:, 170:4928], T1f[:, 172:4930])
                stt(FQf[:, 170:4930], T1f[:, 170:4930], 2.0,
                    T2f[:, 170:4930], Alu.mult, Alu.add)
                g_mm_square(BAND[4], FQ3, dstt, False)
                # gz = derivH(s121D(onesW))
                VA(T1f[:, 1:5099], Xf[:, 0:5098], Xf[:, 2:5100])
                VA(T2f[:, 1:5099], T1f[:, 1:5099], Xf[:, 1:5099])
                VA(FQf[:, 170:4930], T2f[:, 0:4760], T2f[:, 340:5100])
                stt(T1f[:, 170:4930], T2f[:, 170:4930], 2.0,
                    FQf[:, 170:4930], Alu.mult, Alu.add)
                g_mm_square(BAND[5], T13, dstt, False)
                nc.scalar.sqrt(dstt[:, 170:4930], dstt[:, 170:4930])

            for ph, (h0, acc_lo, acc_hi, rlo, rhi) in enumerate(H_TILES):
                band = BAND[ph]
                nc.sync.dma_start(XT3, xt_d[:, h0:h0 + 128, :].transpose([1, 0, 2]))
                nc.sync.dma_start(XP3, xp_d[:, h0:h0 + 128, :].transpose([1, 0, 2]))

                for br in range(3):
                    if br == 0:
                        FAt, FA3, FBt, FB3 = XT, XT3, XP, XP3
                    elif br == 1:
                        build_E(XT, XT3, A, A3)
                        mask_field(A3)
                        zap_halo(A3)
                        build_E(XP, XP3, B, B3)
                        mask_field(B3)
                        zap_halo(B3)
                        FAt, FA3, FBt, FB3 = A, A3, B, B3
                    else:
                        build_S(XT, XT3, A, A3)
                        mask_field(A3)
                        zap_halo(A3)
                        build_S(XP, XP3, B, B3)
                        mask_field(B3)
                        zap_halo(B3)
                        FAt, FA3, FBt, FB3 = A, A3, B, B3

                    box(FAt, band, EV[0][:])                # Is
                    box(FBt, band, EV[1][:])                # Js
                    nc.scalar.square(FQ3[:, 1:29, :], FA3[:, 1:29, :])
                    box(FSQ, band, EV[2][:])                # I2s
                    nc.scalar.square(FQ3[:, 1:29, :], FB3[:, 1:29, :])
                    box(FSQ, band, EV[3][:])                # J2s
                    VM(FQ3[:, 1:29, :], FA3[:, 1:29, :], FB3[:, 1:29, :])
                    box(FSQ, band, EV[4][:])                # IJs

                    # NCC pointwise: fp16 vars, fp32 tail
                    Is, Js, I2s, J2s, IJs = (e[:] for e in EV)
                    S1, S2, Sh = PW1[:], PW2[:], PWH[:]
                    col = ACC[:, ph * 3 + br:ph * 3 + br + 1]
                    nc.scalar.activation(Sh, Is, Act.Square,
                                         scale=1.0 / 27.0)          # n2
                    VS(I2s, I2s, Sh)                                # Ivar
                    nc.scalar.activation(Sh, Js, Act.Square,
                                         scale=1.0 / 27.0)          # n3
                    VS(J2s, J2s, Sh)                                # Jvar
                    stt(Sh, Is, INV_WS, Js, Alu.mult, Alu.mult)     # n1
                    VS(IJs, IJs, Sh)                                # cross
                    stt(S1, I2s, EPS, J2s, Alu.add, Alu.mult)       # den
                    nc.vector.reciprocal_approx_fast(S2, S1)        # rden
                    nc.scalar.activation(S1, IJs, Act.Square)       # num
                    stt(S1, S1, 1.0, S2, Alu.mult, Alu.mult,
                        accum_out=col)

            nc.sync.dma_start(out_d, ACC[:])
    nc.compile()
    return nc


def _get_nc():
    if "nc" not in _CACHE:
        _CACHE["nc"] = _build_program()
    return _CACHE["nc"]


def _host_inputs(y_true, y_pred):
    xt = np.ascontiguousarray(np.asarray(y_true, np.float32).reshape(D, H, W))
    xp = np.ascontiguousarray(np.asarray(y_pred, np.float32).reshape(D, H, W))
    big_t = np.zeros((D + 2 * PAD, HP, WP), np.float16)
    big_p = np.zeros((D + 2 * PAD, HP, WP), np.float16)
    big_t[PAD:PAD + D, PAD:PAD + H, PAD:PAD + W] = xt.astype(np.float16)
    big_p[PAD:PAD + D, PAD:PAD + H, PAD:PAD + W] = xp.astype(np.float16)
    band0 = _make_band(5, 127).astype(np.float16)
    band1 = _make_band(1, 123).astype(np.float16)
    def bmat(taps):
        Bm = np.zeros((128, 128), np.float32)
        for o, t in taps:
            for r in range(128):
                if 0 <= r + o < 128:
                    Bm[r + o, r] += t
        return Bm.astype(np.float16)
    band_lap = bmat([(-1, -1.0), (0, 2.0), (1, -1.0)])
    band_121 = bmat([(-1, 1.0), (0, 2.0), (1, 1.0)])
    band_one = bmat([(-1, 1.0), (0, 1.0), (1, 1.0)])
    band_drv = bmat([(-1, -1.0), (1, 1.0)])
    in_maps = []
    for c in range(N_CORES):
        d0 = c * DS
        mask = np.zeros((128, 32), np.float32)
        for j in range(DP):
            if 0 <= d0 - PAD + j < D:
                mask[:, j] = 1.0
        in_maps.append({
            "xt": np.ascontiguousarray(big_t[d0:d0 + DP]),
            "xp": np.ascontiguousarray(big_p[d0:d0 + DP]),
            "mask": mask,
            "band0": band0,
            "band1": band1,
            "band_lap": band_lap,
            "band_121": band_121,
            "band_one": band_one,
            "band_drv": band_drv,
        })
    return in_maps


def _combine(results):
    total = np.zeros(3, np.float64)
    for res in results:
        cols = np.asarray(res["out"], np.float64)
        for ph, (_, lo, hi, _, _) in enumerate(H_TILES):
            for br in range(3):
                total[br] += cols[lo:hi, ph * 3 + br].sum()
    losses = -total / NVOX
    return np.float32(0.8 * losses[0] + 0.1 * losses[1] + 0.1 * losses[2])


def kernel(y_true, y_pred):
    from concourse.bass_utils import run_bass_kernel_spmd
    nc = _get_nc()
    in_maps = _host_inputs(y_true, y_pred)
    res = run_bass_kernel_spmd(nc, in_maps, core_ids=list(range(N_CORES)))
    return _combine(res.results)


if __name__ == "__main__":
    g = np.load("/root/problem/golden.npz")
    got = float(kernel(g["y_true"], g["y_pred"]))
    exp = float(g["expected"])
    print(f"expected {exp:.9f} got {got:.9f} rel {abs(got-exp)/abs(exp):.3e}")
